# revision 15
# baseline (speedup 1.0000x reference)
"""Trainium2 Bass kernel for nn_MixingBlock (local-window attention + MLP).

Sharding: 8 cores = (batch 0..3) x (token half 0..1); each core computes
1024 output tokens of one batch element. Attention is local (7x11 window
on the 32x64 token grid), so each core works on a zero-padded 22-image-row
slab (T=1408 tokens) of x and needs no collectives: slab rows for half h
are global rows [16h-3, 16h+19), zero-padded outside [0,32). Queries sit
at slab tokens [192, 1216) and the key window of query tile i is slab
tokens [128i, 128i+512) for BOTH halves, so one SPMD program serves all
cores; the {0,1} attention mask (built host-side from the true mask)
kills padded and out-of-window keys.

Device layouts: Q^T/K^T channels-on-partitions ([512f, T], Q pre-scaled),
V token-major with a ones column per head ([T, 33*8]); scores are built
transposed (S^T = K Q^T) so softmax weights can feed the AV matmul as
stationary operands without transposes; the appended ones column yields
softmax denominators inside the same accumulation. MLP/LN run token-major
(bn_stats over channels) with one PE transpose of z1. ln1_g / biases are
folded into weights host-side where linear algebra allows.
"""

import contextlib
import sys
import types

import ml_dtypes
import numpy as np

import concourse.bass as bass
import concourse.mybir as mybir
import concourse.tile as tile

# ---------------------------------------------------------------------------
# axon NTFF profile hook (lets run_bass_kernel_spmd(trace=True) work here)
# ---------------------------------------------------------------------------
if "antenv.axon_hooks" not in sys.modules:
    try:
        import antenv  # noqa: F401

        _hookmod = types.ModuleType("antenv.axon_hooks")
        _hookmod._hook = None
        _hookmod.set_axon_ntff_profile_hook = lambda h: setattr(_hookmod, "_hook", h)
        _hookmod.get_axon_ntff_profile_hook = lambda: _hookmod._hook
        sys.modules["antenv.axon_hooks"] = _hookmod
        try:
            from trn_agent_boot.trn_boot import _ntff_profile_via_ctypes

            _hookmod.set_axon_ntff_profile_hook(
                _ntff_profile_via_ctypes("/opt/axon/libaxon_pjrt.so")
            )
        except Exception:
            pass
    except Exception:
        pass

from concourse.bass_utils import run_bass_kernel_spmd  # noqa: E402
from concourse.masks import make_identity  # noqa: E402

F32 = mybir.dt.float32
BF16 = mybir.dt.bfloat16
AF = mybir.ActivationFunctionType
ALU = mybir.AluOpType

# Problem constants
H_IMG, W_IMG = 32, 64
N = H_IMG * W_IMG  # 2048
C = 256
NH = 8
HD = 32
HIDDEN = 1024
SCALE = HD**-0.5
B = 4
LN_EPS = 1e-5

# Sharding constants
PAD_ROWS = 3
SLAB_ROWS = 16 + 2 * PAD_ROWS  # 22
T = SLAB_ROWS * W_IMG  # 1408
NQ = 1024
NQT = 8
KW = 512  # key window tokens per query tile
Q0 = PAD_ROWS * W_IMG  # 192: first query token within slab

MAX_WAITS = 1


def _split_excess_waits(nc):
    """walrus accepts only MAX_WAITS sem-waits per instruction; move excess
    onto same-engine nops inserted immediately before the instruction."""
    for f in nc.m.functions:
        for bb in f.blocks:
            i = 0
            while i < len(bb.instructions):
                ins = bb.instructions[i]
                si = ins.sync_info
                if si is not None and si.on_wait and len(si.on_wait) > MAX_WAITS:
                    waits = list(si.on_wait)
                    extra, keep = waits[:-MAX_WAITS], waits[-MAX_WAITS:]
                    ins.sync_info = mybir.SyncInfo(
                        on_wait=keep, on_update=list(si.on_update)
                    )
                    nops = []
                    for j in range(0, len(extra), MAX_WAITS):
                        nop = nc.engines[ins.engine].nop().ins
                        cur = nc.cur_bb.bb
                        assert cur.instructions[-1] is nop
                        cur.instructions.pop()
                        nop.sync_info = mybir.SyncInfo(
                            on_wait=extra[j : j + MAX_WAITS], on_update=[]
                        )
                        nops.append(nop)
                    bb.instructions[i:i] = nops
                    i += len(nops)
                i += 1


def _build_nc():
    nc = bass.Bass("TRN2", target_bir_lowering=False, num_devices=8)

    d = {}
    d["xT"] = nc.dram_tensor("xT", [C, T], BF16, kind="ExternalInput")
    d["xres"] = nc.dram_tensor("xres", [NQ, C], F32, kind="ExternalInput")
    d["wqk"] = nc.dram_tensor("wqk", [C, 512], BF16, kind="ExternalInput")
    d["qkb"] = nc.dram_tensor("qkb", [512, 1], F32, kind="ExternalInput")
    d["wv"] = nc.dram_tensor("wv", [C, 264], BF16, kind="ExternalInput")
    d["wp"] = nc.dram_tensor("wp", [C, C], BF16, kind="ExternalInput")
    d["w1"] = nc.dram_tensor("w1", [C, HIDDEN], BF16, kind="ExternalInput")
    d["b1"] = nc.dram_tensor("b1", [HIDDEN, 1], F32, kind="ExternalInput")
    d["w2"] = nc.dram_tensor("w2", [HIDDEN, C], BF16, kind="ExternalInput")
    d["rows"] = nc.dram_tensor("rows", [1, 3 * C], F32, kind="ExternalInput")
    d["bv2"] = nc.dram_tensor("bv2", [1, C], BF16, kind="ExternalInput")
    d["mask"] = nc.dram_tensor("mask", [NQT, 128, KW], BF16, kind="ExternalInput")
    d["out"] = nc.dram_tensor("out", [NQ, C], F32, kind="ExternalOutput")

    with tile.TileContext(nc) as tc:
        _emit(nc, tc, d)

    _split_excess_waits(nc)
    return nc


def _emit(nc, tc, d):
    ctx = contextlib.ExitStack()
    with ctx:
        const = ctx.enter_context(tc.tile_pool(name="const", bufs=1))
        big = ctx.enter_context(tc.tile_pool(name="big", bufs=1))
        ps = ctx.enter_context(tc.tile_pool(name="ps", bufs=2, space="PSUM"))
        ps_s = ctx.enter_context(tc.tile_pool(name="ps_s", bufs=3, space="PSUM"))
        work = ctx.enter_context(tc.tile_pool(name="work", bufs=3))
        small = ctx.enter_context(tc.tile_pool(name="small", bufs=4))

        # ---------------- inputs to SBUF (issue order = first use) -------
        xT = [const.tile([128, T], BF16, tag=f"xT{i}", name=f"xT{i}") for i in range(2)]
        wqk = [const.tile([128, 512], BF16, tag=f"wqk{i}", name=f"wqk{i}") for i in range(2)]
        wv = [const.tile([128, 264], BF16, tag=f"wv{i}", name=f"wv{i}") for i in range(2)]
        wp = [const.tile([128, C], BF16, tag=f"wp{i}", name=f"wp{i}") for i in range(2)]
        w1 = [const.tile([128, HIDDEN], BF16, tag=f"w1{i}", name=f"w1s{i}") for i in range(2)]
        qkb = [const.tile([128, 1], F32, tag=f"qkb{i}", name=f"qkb{i}") for i in range(4)]
        b1 = [const.tile([128, 1], F32, tag=f"b1{i}", name=f"b1s{i}") for i in range(8)]
        w2 = [const.tile([128, C], BF16, tag=f"w2{i}", name=f"w2s{i}") for i in range(8)]
        xres = [const.tile([128, C], F32, tag=f"xres{i}", name=f"xres{i}") for i in range(8)]
        mask = [const.tile([128, KW], BF16, tag=f"mask{i}", name=f"mask{i}") for i in range(NQT)]
        for i in range(2):
            nc.sync.dma_start(out=xT[i][:], in_=d["xT"][128 * i : 128 * (i + 1), :])
        for i in range(2):
            nc.sync.dma_start(out=wqk[i][:], in_=d["wqk"][128 * i : 128 * (i + 1), :])
        for i in range(4):
            nc.sync.dma_start(out=qkb[i][:], in_=d["qkb"][128 * i : 128 * (i + 1), :])
        for i in range(2):
            nc.sync.dma_start(out=wv[i][:], in_=d["wv"][128 * i : 128 * (i + 1), :])
        for i in range(NQT):
            nc.sync.dma_start(out=mask[i][:], in_=d["mask"][i])
        for i in range(2):
            nc.sync.dma_start(out=wp[i][:], in_=d["wp"][128 * i : 128 * (i + 1), :])
        for i in range(8):
            nc.sync.dma_start(
                out=xres[i][:], in_=d["xres"][128 * i : 128 * (i + 1), :]
            )
        for i in range(2):
            nc.sync.dma_start(out=w1[i][:], in_=d["w1"][128 * i : 128 * (i + 1), :])
        for i in range(8):
            nc.sync.dma_start(out=b1[i][:], in_=d["b1"][128 * i : 128 * (i + 1), :])
        for i in range(8):
            nc.sync.dma_start(out=w2[i][:], in_=d["w2"][128 * i : 128 * (i + 1), :])
        bv2 = const.tile([1, C], BF16)
        nc.sync.dma_start(out=bv2[:], in_=d["bv2"][:])
        g1b = const.tile([128, C], F32)
        g2b = const.tile([128, C], F32)
        b2lb = const.tile([128, C], F32)
        nc.gpsimd.dma_start(
            out=g1b[:], in_=d["rows"][0:1, 0:C].to_broadcast((128, C))
        )
        nc.gpsimd.dma_start(
            out=g2b[:], in_=d["rows"][0:1, C : 2 * C].to_broadcast((128, C))
        )
        nc.gpsimd.dma_start(
            out=b2lb[:], in_=d["rows"][0:1, 2 * C : 3 * C].to_broadcast((128, C))
        )

        eps_t = const.tile([128, 1], F32)
        nc.vector.memset(eps_t[:], LN_EPS)
        ones_col = const.tile([1, 128], BF16)
        nc.vector.memset(ones_col[:], 1.0)
        ident = const.tile([128, 128], BF16)
        make_identity(nc, ident)

        # ---------------- phase 1: Q^T (scaled) and K^T ----------------
        qkT = [big.tile([128, T], BF16, tag=f"qkT{m}", name=f"qkT{m}") for m in range(4)]
        for m in range(4):
            for off in range(0, T, 512):
                w = min(512, T - off)
                p = ps.tile([128, 512], F32, tag="mm", name="p_qk")
                for cc in range(2):
                    nc.tensor.matmul(
                        p[:, :w],
                        wqk[cc][:, 128 * m : 128 * (m + 1)],
                        xT[cc][:, off : off + w],
                        start=(cc == 0),
                        stop=(cc == 1),
                    )
                nc.scalar.activation(
                    out=qkT[m][:, off : off + w],
                    in_=p[:, :w],
                    func=AF.Identity,
                    bias=qkb[m][:],
                    scale=1.0,
                )

        # ---------------- phase 2: V (token-major, ones columns) ----------
        vt = [big.tile([128, 264], BF16, tag=f"vt{i}", name=f"vt{i}") for i in range(T // 128)]
        for i in range(T // 128):
            p = ps.tile([128, 264], F32, tag="mm", name="p_v")
            for cc in range(2):
                nc.tensor.matmul(
                    p[:, :264],
                    xT[cc][:, 128 * i : 128 * (i + 1)],
                    wv[cc][:],
                    start=(cc == 0),
                    stop=(cc == 1),
                )
            nc.vector.tensor_copy(vt[i][:], p[:, :264])
            for h in range(NH):
                nc.gpsimd.memset(vt[i][:, 33 * h + 32 : 33 * h + 33], 1.0)

        # ---------------- phase 3: attention ----------------
        attnT = [
            [
                big.tile([128, 128], BF16, tag=f"attnT{j}_{q}", name=f"attnT{j}_{q}")
                for q in range(NQT)
            ]
            for j in range(2)
        ]
        for qt in range(NQT):
            kw0 = 128 * qt  # key window start token in slab
            attn_q = work.tile([128, C], BF16, tag="attn_q", name="attn_q")
            for hq in range(2):  # two groups of 4 heads
                heads = [4 * hq + j for j in range(4)]
                p_sA = ps_s.tile([128, 2, KW], F32, tag="s_ps", name="p_sA")
                p_sB = ps_s.tile([128, 2, KW], F32, tag="s_ps", name="p_sB")
                p_of = {heads[0]: (p_sA, 0), heads[1]: (p_sA, 1),
                        heads[2]: (p_sB, 0), heads[3]: (p_sB, 1)}
                for c in range(4):
                    for h in heads:
                        pt_, hi = p_of[h]
                        ktile, koff = 2 + h // 4, (32 * h) % 128
                        qtile, qoff = h // 4, (32 * h) % 128
                        nc.tensor.matmul(
                            pt_[:, hi, 128 * c : 128 * (c + 1)],
                            qkT[ktile][
                                koff : koff + 32,
                                kw0 + 128 * c : kw0 + 128 * (c + 1),
                            ],
                            qkT[qtile][
                                qoff : qoff + 32,
                                Q0 + 128 * qt : Q0 + 128 * (qt + 1),
                            ],
                            start=True,
                            stop=True,
                            tile_position=(koff, 0),
                        )
                for pi, p_s in enumerate((p_sA, p_sB)):
                    pT = work.tile([128, 2, KW], BF16, tag="pT", name="pT")
                    nc.scalar.activation(
                        out=pT[:], in_=p_s[:], func=AF.Exp, bias=0.0, scale=1.0
                    )
                    eng = nc.vector if pi == 0 else nc.gpsimd
                    for hi in range(2):
                        eng.tensor_mul(pT[:, hi, :], pT[:, hi, :], mask[qt][:])
                    for hi in range(2):
                        h = heads[2 * pi + hi]
                        p_av = ps.tile([128, 33], F32, tag="mm", name="p_av")
                        for c in range(4):
                            nc.tensor.matmul(
                                p_av[:, :33],
                                pT[:, hi, 128 * c : 128 * (c + 1)],
                                vt[qt + c][:, 33 * h : 33 * h + 33],
                                start=(c == 0),
                                stop=(c == 3),
                            )
                        rec = small.tile([128, 1], F32, tag="rec")
                        nc.vector.reciprocal(rec[:], p_av[:, 32:33])
                        nc.vector.tensor_scalar_mul(
                            out=attn_q[:, 32 * h : 32 * h + 32],
                            in0=p_av[:, 0:32],
                            scalar1=rec[:],
                        )
            for j in range(2):
                p_t2 = ps.tile([128, 128], BF16, tag="mm", name="p_t2")
                nc.tensor.transpose(
                    p_t2[:, :128], attn_q[:, 128 * j : 128 * (j + 1)], ident[:]
                )
                nc.vector.tensor_copy(attnT[j][qt][:], p_t2[:, :128])

        # ---------------- phase 4: proj + residual + LN1 ----------------
        z1 = [big.tile([128, C], F32, tag=f"z1{i}", name=f"z1_{i}") for i in range(8)]
        z1bf = [big.tile([128, C], BF16, tag=f"z1bf{i}", name=f"z1bf{i}") for i in range(8)]
        for t in range(8):
            p_p = ps.tile([128, C], F32, tag="mm", name="p_p")
            for cc in range(2):
                nc.tensor.matmul(
                    p_p[:, :C],
                    attnT[cc][t][:],
                    wp[cc][:],
                    start=(cc == 0),
                    stop=(cc == 1),
                )
            r1 = work.tile([128, C], F32, tag="r1")
            nc.vector.tensor_add(r1[:], p_p[:, :C], xres[t][:])
            stats = small.tile([128, 6], F32, tag="stats")
            nc.vector.bn_stats(out=stats[:], in_=r1[:])
            mv = small.tile([128, 2], F32, tag="mv")
            nc.vector.bn_aggr(out=mv[:], in_=stats[:])
            lnv = small.tile([128, 1], F32, tag="lnv")
            nc.scalar.activation(
                out=lnv[:], in_=mv[:, 1:2], func=AF.Ln, bias=eps_t[:], scale=1.0
            )
            rstd = small.tile([128, 1], F32, tag="rstd")
            nc.scalar.activation(
                out=rstd[:], in_=lnv[:], func=AF.Exp, bias=0.0, scale=-0.5
            )
            nc.vector.tensor_scalar(
                out=z1[t][:],
                in0=r1[:],
                scalar1=mv[:, 0:1],
                scalar2=rstd[:],
                op0=ALU.subtract,
                op1=ALU.mult,
            )
            nc.vector.tensor_copy(z1bf[t][:], z1[t][:])

        # ---------------- phase 4b: z1^T (PE transpose) ----------------
        z1T = [
            [
                big.tile([128, 512], BF16, tag=f"z1T{j}_{p}", name=f"z1T{j}_{p}")
                for p in range(2)
            ]
            for j in range(2)
        ]
        for t in range(8):
            for j in range(2):
                p_t = ps.tile([128, 128], BF16, tag="mm", name="p_t")
                nc.tensor.transpose(
                    p_t[:, :128], z1bf[t][:, 128 * j : 128 * (j + 1)], ident[:]
                )
                nc.vector.tensor_copy(
                    z1T[j][t // 4][:, 128 * (t % 4) : 128 * (t % 4 + 1)],
                    p_t[:, :128],
                )

        # ---------------- phase 5: mlp1 + gelu (hidden-major) ------------
        hT = [
            [
                big.tile([128, 512], BF16, tag=f"hT{i}_{p}", name=f"hT{i}_{p}")
                for p in range(2)
            ]
            for i in range(8)
        ]
        for piece in range(2):
            for hc in range(8):
                p_h = ps_s.tile([128, 512], F32, tag="s_ps", name="p_h")
                for cc in range(2):
                    nc.tensor.matmul(
                        p_h[:, :512],
                        w1[cc][:, 128 * hc : 128 * (hc + 1)],
                        z1T[cc][piece][:],
                        start=(cc == 0),
                        stop=(cc == 1),
                    )
                nc.scalar.activation(
                    out=hT[hc][piece][:],
                    in_=p_h[:, :512],
                    func=AF.Gelu,
                    bias=b1[hc][:],
                    scale=1.0,
                )

        # ---------------- phase 6: mlp2 + resid2 + LN2 + out -------------
        for t in range(8):
            p_m = ps.tile([128, C], F32, tag="mm", name="p_m")
            nc.tensor.matmul(p_m[:, :C], ones_col[:], bv2[:], start=True, stop=False)
            for hc in range(8):
                nc.tensor.matmul(
                    p_m[:, :C],
                    hT[hc][t // 4][:, 128 * (t % 4) : 128 * (t % 4 + 1)],
                    w2[hc][:],
                    start=False,
                    stop=(hc == 7),
                )
            r2 = work.tile([128, C], F32, tag="r2")
            nc.gpsimd.tensor_mul(r2[:], z1[t][:], g1b[:])
            nc.vector.tensor_add(r2[:], r2[:], p_m[:, :C])
            stats = small.tile([128, 6], F32, tag="stats2")
            nc.vector.bn_stats(out=stats[:], in_=r2[:])
            mv = small.tile([128, 2], F32, tag="mv2")
            nc.vector.bn_aggr(out=mv[:], in_=stats[:])
            lnv = small.tile([128, 1], F32, tag="lnv2")
            nc.scalar.activation(
                out=lnv[:], in_=mv[:, 1:2], func=AF.Ln, bias=eps_t[:], scale=1.0
            )
            rstd = small.tile([128, 1], F32, tag="rstd2")
            nc.scalar.activation(
                out=rstd[:], in_=lnv[:], func=AF.Exp, bias=0.0, scale=-0.5
            )
            z2 = work.tile([128, C], F32, tag="z2")
            nc.vector.tensor_scalar(
                out=z2[:],
                in0=r2[:],
                scalar1=mv[:, 0:1],
                scalar2=rstd[:],
                op0=ALU.subtract,
                op1=ALU.mult,
            )
            o = work.tile([128, C], F32, tag="o")
            nc.vector.tensor_mul(o[:], z2[:], g2b[:])
            nc.gpsimd.tensor_add(o[:], o[:], b2lb[:])
            nc.sync.dma_start(out=d["out"][128 * t : 128 * (t + 1), :], in_=o[:])


_NC_CACHE = None
_LAST_RESULT = None


def _get_nc():
    global _NC_CACHE
    if _NC_CACHE is None:
        _NC_CACHE = _build_nc()
    return _NC_CACHE


def _to_bf16(a):
    return np.ascontiguousarray(np.asarray(a, dtype=np.float32)).astype(
        ml_dtypes.bfloat16
    )


def _host_inputs(core, x, mask, qkv_w, qkv_b, proj_w, proj_b, ln1_g, ln1_b, w1,
                 b1, w2, b2, ln2_g, ln2_b):
    b = core // 2
    half = core % 2
    row0 = 16 * half - PAD_ROWS  # slab start image row (may be negative)
    S0 = row0 * W_IMG  # slab start token
    Q0g = 1024 * half  # first query token (global)

    xb = np.asarray(x[b], dtype=np.float32)  # [N, C]
    slab = np.zeros((T, C), np.float32)
    g_lo, g_hi = max(0, S0), min(N, S0 + T)
    slab[g_lo - S0 : g_hi - S0] = xb[g_lo:g_hi]

    wqk = np.concatenate([qkv_w[:C] * SCALE, qkv_w[C : 2 * C]], axis=0)  # [512,C]
    qkb = np.concatenate([qkv_b[:C] * SCALE, qkv_b[C : 2 * C]])[:, None]
    wv = qkv_w[2 * C :]  # [256, 256]
    vb = qkv_b[2 * C :]
    assert np.abs(vb).max() == 0.0, "nonzero v bias not folded"
    wv_pad = np.zeros((C, 264), np.float32)
    for h in range(NH):
        wv_pad[:, 33 * h : 33 * h + 32] = wv[32 * h : 32 * h + 32].T

    w1f = w1 * ln1_g[None, :]  # fold ln1 gamma
    b1f = (b1 + w1 @ ln1_b)[:, None]  # fold ln1 beta (mlp path)
    bvec2 = b2 + ln1_b  # resid2 constant (residual path)

    xres = xb[Q0g : Q0g + NQ] + proj_b[None, :]

    mtiles = np.zeros((NQT, 128, KW), np.float32)
    for i in range(NQT):
        qg = Q0g + 128 * i
        valid = np.zeros((128, KW), np.float32)  # [q, k-in-window]
        for r in range(8):
            gr = row0 + 2 * i + r  # global image row of window row r
            if 0 <= gr < H_IMG:
                valid[:, 64 * r : 64 * (r + 1)] = (
                    mask[qg : qg + 128, 64 * gr : 64 * (gr + 1)] == 0
                )
        # coverage check: every allowed key lies inside the window
        full = mask[qg : qg + 128] == 0
        assert int(full.sum()) == int(valid.sum()), (core, i, "window coverage")
        # m[p, 128c+q] = valid[q, 128c+p]
        mtiles[i] = (
            valid.T.reshape(4, 128, 128).transpose(1, 0, 2).reshape(128, KW)
        )

    rows = np.concatenate([ln1_g, ln2_g, ln2_b])[None, :]

    return {
        "xT": _to_bf16(slab.T),
        "xres": np.ascontiguousarray(xres, dtype=np.float32),
        "wqk": _to_bf16(wqk.T),
        "qkb": np.ascontiguousarray(qkb, dtype=np.float32),
        "wv": _to_bf16(wv_pad),
        "wp": _to_bf16(proj_w.T),
        "w1": _to_bf16(w1f.T),
        "b1": np.ascontiguousarray(b1f, dtype=np.float32),
        "w2": _to_bf16(w2.T),
        "rows": np.ascontiguousarray(rows, dtype=np.float32),
        "bv2": _to_bf16(bvec2[None, :]),
        "mask": _to_bf16(mtiles),
    }


def kernel(**inputs):
    args = {k: np.asarray(v) for k, v in inputs.items()}
    in_maps = [
        _host_inputs(
            core,
            args["x"],
            np.asarray(args["mask"], dtype=np.float32),
            args["qkv_w"],
            args["qkv_b"],
            args["proj_w"],
            args["proj_b"],
            args["ln1_g"],
            args["ln1_b"],
            args["w1"],
            args["b1"],
            args["w2"],
            args["b2"],
            args["ln2_g"],
            args["ln2_b"],
        )
        for core in range(8)
    ]
    nc = _get_nc()
    res = run_bass_kernel_spmd(nc, in_maps, core_ids=list(range(8)))
    global _LAST_RESULT
    _LAST_RESULT = res
    out = np.zeros((B, N, C), np.float32)
    for core in range(8):
        b, half = core // 2, core % 2
        out[b, 1024 * half : 1024 * (half + 1)] = res.results[core]["out"]
    return out
